# revision 1
# baseline (speedup 1.0000x reference)
"""Expert-parallel MoE kernel for Trainium2 (8 NeuronCores).

Strategy (matches the expert-parallel sharding hint):
  - Router is evaluated on host with the exact same jax ops as the
    reference (same backend) so top-k decisions match bit-for-bit.
  - Tokens are dispatched (gathered) per expert on host; each of the 8
    cores owns one expert's weights and runs a fused MLP
        Y = (silu(X @ G^T) * (X @ U^T)) @ D^T
    over its gathered tokens in bf16 (fp32 PSUM accumulate).
  - Outputs are combined on host: out[token] += mean_w[e] * Y_e[row].
"""

import sys

if "/opt/trn_rl_repo" not in sys.path:
    sys.path.insert(0, "/opt/trn_rl_repo")

import ml_dtypes
import numpy as np

import concourse.bacc as bacc
import concourse.mybir as mybir
import concourse.tile as tile
from concourse.bass_utils import run_bass_kernel_spmd

B, S, H, I, E, TOPK = 4, 2048, 1024, 4096, 8, 2
T = B * S
KCH = H // 128   # 8 contraction chunks over H
IB = I // 128    # 32 blocks over I
BF16 = mybir.dt.bfloat16
F32 = mybir.dt.float32

_prog_cache: dict[int, object] = {}


def _ctiles(C):
    """Split capacity C into free-dim tiles of 512 (tail multiple of 128)."""
    out = []
    c = 0
    while c < C:
        s = min(512, C - c)
        out.append((c, s))
        c += s
    return out


def build_program(C):
    if C in _prog_cache:
        return _prog_cache[C]
    nc = bacc.Bacc("TRN2", target_bir_lowering=False, debug=False, num_devices=8)

    xt_d = nc.dram_tensor("xt", [KCH, 128, C], BF16, kind="ExternalInput").ap()
    gt_d = nc.dram_tensor("gt", [IB, 128, KCH, 128], BF16, kind="ExternalInput").ap()
    ut_d = nc.dram_tensor("ut", [IB, 128, KCH, 128], BF16, kind="ExternalInput").ap()
    dt_d = nc.dram_tensor("dt", [IB, 128, H], BF16, kind="ExternalInput").ap()
    y_d = nc.dram_tensor("y", [C, H], F32, kind="ExternalOutput").ap()

    with tile.TileContext(nc) as tc:
        with (
            tc.tile_pool(name="wpool", bufs=3) as wpool,
            tc.tile_pool(name="xpool", bufs=2) as xpool,
            tc.tile_pool(name="dpool", bufs=1) as dpool,
            tc.tile_pool(name="hpool", bufs=2) as hpool,
            tc.tile_pool(name="spool", bufs=3) as spool,
            tc.tile_pool(name="ypool", bufs=3) as ypool,
            tc.tile_pool(name="psum", bufs=2, space="PSUM") as psum,
        ):
            # D^T resident in SBUF for the whole kernel: 32 x [128, 1024] bf16
            dts = []
            for ic in range(IB):
                dt = dpool.tile([128, H], BF16, tag=f"dt{ic}")
                nc.sync.dma_start(dt[:], dt_d[ic])
                dts.append(dt)

            for c0, cs in _ctiles(C):
                # load X^T k-chunks for this token tile
                xts = []
                for k in range(KCH):
                    xt = xpool.tile([128, cs], BF16, tag=f"xt{k}")
                    nc.sync.dma_start(xt[:], xt_d[k][:, c0 : c0 + cs])
                    xts.append(xt)

                # stage 1: Hh^T[i_block, c] = silu(G X) * (U X)
                hhs = []
                for ib in range(IB):
                    gt = wpool.tile([128, KCH, 128], BF16, tag="gt")
                    nc.sync.dma_start(gt[:], gt_d[ib])
                    ut = wpool.tile([128, KCH, 128], BF16, tag="ut")
                    nc.sync.dma_start(ut[:], ut_d[ib])

                    a1 = psum.tile([128, cs], F32, tag="a1")
                    for k in range(KCH):
                        nc.tensor.matmul(
                            a1[:], gt[:, k, :], xts[k][:],
                            start=(k == 0), stop=(k == KCH - 1),
                        )
                    a2 = psum.tile([128, cs], F32, tag="a2")
                    for k in range(KCH):
                        nc.tensor.matmul(
                            a2[:], ut[:, k, :], xts[k][:],
                            start=(k == 0), stop=(k == KCH - 1),
                        )
                    sl = spool.tile([128, cs], F32, tag="silu")
                    nc.scalar.activation(
                        sl[:], a1[:], mybir.ActivationFunctionType.Silu
                    )
                    hh = hpool.tile([128, cs], BF16, tag=f"hh{ib}")
                    nc.vector.tensor_mul(hh[:], sl[:], a2[:])
                    hhs.append(hh)

                # stage 2: Y[c, h] = Hh @ D^T  (contract I)
                for h0 in range(0, H, 512):
                    for cs0 in range(0, cs, 128):
                        w = min(128, cs - cs0)
                        py = psum.tile([w, 512], F32, tag="py")
                        for ic in range(IB):
                            nc.tensor.matmul(
                                py[:],
                                hhs[ic][:, cs0 : cs0 + w],
                                dts[ic][:, h0 : h0 + 512],
                                start=(ic == 0), stop=(ic == IB - 1),
                            )
                        yt = ypool.tile([w, 512], F32, tag="yt")
                        nc.scalar.copy(yt[:], py[:])
                        nc.sync.dma_start(
                            y_d[c0 + cs0 : c0 + cs0 + w, h0 : h0 + 512], yt[:]
                        )

    nc.compile()
    _prog_cache[C] = nc
    return nc


def _routing(x, router_w):
    """Replicate the reference's routing decisions with identical jax ops."""
    import jax
    import jax.numpy as jnp

    xf = jnp.asarray(x).reshape(-1, H)
    logits = xf @ jnp.asarray(router_w).T
    probs = jax.nn.softmax(logits, axis=-1)
    topk_p, topk_i = jax.lax.top_k(probs, TOPK)
    topk_p = topk_p / topk_p.sum(axis=-1, keepdims=True)
    return np.asarray(topk_p), np.asarray(topk_i)


def prepare(x, router_w, gate_w, up_w, down_w):
    """Host-side dispatch: returns (nc, in_maps, combine) where combine maps
    the per-core device outputs to the full [B,S,H] result."""
    topk_p, topk_i = _routing(x, router_w)
    xf = np.ascontiguousarray(np.asarray(x, dtype=np.float32).reshape(T, H))

    idxs, weights = [], []
    for e in range(E):
        sel = topk_i == e
        mask = sel.any(axis=-1)
        w_tok = (topk_p * sel).sum(axis=-1)
        cnt = int(mask.sum())
        mean_w = float(w_tok.sum() / max(cnt, 1)) if cnt > 0 else 0.0
        idxs.append(np.nonzero(mask)[0])
        weights.append(np.float32(mean_w))

    cmax = max(len(ix) for ix in idxs)
    C = ((cmax + 127) // 128) * 128

    xf_bf = xf.astype(ml_dtypes.bfloat16)
    in_maps = []
    for e in range(E):
        ix = idxs[e]
        # X^T packed as [KCH, 128, C]: partition p of chunk k holds row h=k*128+p
        xt = np.zeros((KCH, 128, C), dtype=ml_dtypes.bfloat16)
        xt[:, :, : len(ix)] = xf_bf[ix].T.reshape(KCH, 128, len(ix))
        # G/U packed as [IB, 128(p over H%128? no: p over H), KCH, 128(i)]
        # G^T is [H, I]; tile (ib): [H, 128] -> [128(p), KCH(k), 128(i)] with h = k*128+p
        gT = np.asarray(gate_w[e], dtype=np.float32).T.astype(ml_dtypes.bfloat16)
        uT = np.asarray(up_w[e], dtype=np.float32).T.astype(ml_dtypes.bfloat16)
        gt = np.ascontiguousarray(
            gT.reshape(KCH, 128, IB, 128).transpose(2, 1, 0, 3)
        )
        ut = np.ascontiguousarray(
            uT.reshape(KCH, 128, IB, 128).transpose(2, 1, 0, 3)
        )
        # D^T is [I, H]; packed [IB, 128(p over I), H] with i = ic*128+p
        dT = np.asarray(down_w[e], dtype=np.float32).T.astype(ml_dtypes.bfloat16)
        dt = np.ascontiguousarray(dT.reshape(IB, 128, H))
        in_maps.append({"xt": xt, "gt": gt, "ut": ut, "dt": dt})

    nc = build_program(C)

    def combine(results):
        out = np.zeros((T, H), dtype=np.float32)
        for e in range(E):
            ix = idxs[e]
            y = results[e]["y"]
            out[ix] += weights[e] * y[: len(ix)]
        return out.reshape(B, S, H)

    return nc, in_maps, combine


def kernel(x, router_w, gate_w, up_w, down_w):
    nc, in_maps, combine = prepare(x, router_w, gate_w, up_w, down_w)
    res = run_bass_kernel_spmd(nc, in_maps, list(range(8)))
    return combine(res.results)


# revision 3
# speedup vs baseline: 115.6131x; 115.6131x over previous
"""Expert-parallel MoE kernel for Trainium2 (8 NeuronCores).

Strategy (matches the expert-parallel sharding hint):
  - Router is evaluated on host with the exact same jax ops as the
    reference (same backend) so top-k decisions match bit-for-bit.
  - Tokens are dispatched (gathered) per expert on host; each of the 8
    cores owns one expert's weights and runs a fused MLP
        Y = (silu(X @ G^T) * (X @ U^T)) @ D^T
    over its gathered tokens in bf16 (fp32 PSUM accumulate).
  - Outputs are combined on host: out[token] += mean_w[e] * Y_e[row].
"""

import sys
from contextlib import ExitStack

if "/opt/trn_rl_repo" not in sys.path:
    sys.path.insert(0, "/opt/trn_rl_repo")

import ml_dtypes
import numpy as np

import concourse.bacc as bacc
import concourse.mybir as mybir
import concourse.tile as tile
from concourse.bass_utils import run_bass_kernel_spmd

B, S, H, I, E, TOPK = 4, 2048, 1024, 4096, 8, 2
T = B * S
KCH = H // 128   # 8 contraction chunks over H
IB = I // 128    # 32 blocks over I
BF16 = mybir.dt.bfloat16
F32 = mybir.dt.float32

_prog_cache: dict[int, object] = {}


def _ctiles(C):
    """Split capacity C into free-dim tiles of 512 (tail multiple of 128)."""
    out = []
    c = 0
    while c < C:
        s = min(512, C - c)
        out.append((c, s))
        c += s
    return out


def build_program(C, reps=1):
    key = (C, reps)
    if key in _prog_cache:
        return _prog_cache[key]
    nc = bacc.Bacc("TRN2", target_bir_lowering=False, debug=False, num_devices=8)

    xt_d = nc.dram_tensor("xt", [KCH, 128, C], BF16, kind="ExternalInput").ap()
    gt_d = nc.dram_tensor("gt", [IB, 128, KCH, 128], BF16, kind="ExternalInput").ap()
    ut_d = nc.dram_tensor("ut", [IB, 128, KCH, 128], BF16, kind="ExternalInput").ap()
    dt_d = nc.dram_tensor("dt", [IB, 128, H], BF16, kind="ExternalInput").ap()
    y_d = nc.dram_tensor("y", [C, H], F32, kind="ExternalOutput").ap()

    with tile.TileContext(nc) as tc:
        with ExitStack() as stack:
            if reps > 1:
                stack.enter_context(tc.For_i(0, reps, 1))
            _emit_body(nc, tc, stack, C, xt_d, gt_d, ut_d, dt_d, y_d)

    nc.compile()
    _prog_cache[key] = nc
    return nc


def _emit_body(nc, tc, stack, C, xt_d, gt_d, ut_d, dt_d, y_d):
    with (
        tc.tile_pool(name="wpool", bufs=3) as wpool,
        tc.tile_pool(name="xpool", bufs=2) as xpool,
        tc.tile_pool(name="dpool", bufs=1) as dpool,
        tc.tile_pool(name="hpool", bufs=2) as hpool,
        tc.tile_pool(name="spool", bufs=3) as spool,
        tc.tile_pool(name="ypool", bufs=3) as ypool,
        tc.tile_pool(name="psum", bufs=2, space="PSUM") as psum,
    ):
            # D^T resident in SBUF for the whole kernel: 32 x [128, 1024] bf16
            dts = []
            for ic in range(IB):
                dt = dpool.tile([128, H], BF16, tag=f"dt{ic}")
                nc.sync.dma_start(dt[:], dt_d[ic])
                dts.append(dt)

            for c0, cs in _ctiles(C):
                # load X^T k-chunks for this token tile
                xts = []
                for k in range(KCH):
                    xt = xpool.tile([128, cs], BF16, tag=f"xt{k}")
                    nc.sync.dma_start(xt[:], xt_d[k][:, c0 : c0 + cs])
                    xts.append(xt)

                # stage 1: Hh^T[i_block, c] = silu(G X) * (U X)
                hhs = []
                for ib in range(IB):
                    gt = wpool.tile([128, KCH, 128], BF16, tag="gt")
                    nc.sync.dma_start(gt[:], gt_d[ib])
                    ut = wpool.tile([128, KCH, 128], BF16, tag="ut")
                    nc.sync.dma_start(ut[:], ut_d[ib])

                    a1 = psum.tile([128, cs], F32, tag="a1")
                    for k in range(KCH):
                        nc.tensor.matmul(
                            a1[:], gt[:, k, :], xts[k][:],
                            start=(k == 0), stop=(k == KCH - 1),
                        )
                    a2 = psum.tile([128, cs], F32, tag="a2")
                    for k in range(KCH):
                        nc.tensor.matmul(
                            a2[:], ut[:, k, :], xts[k][:],
                            start=(k == 0), stop=(k == KCH - 1),
                        )
                    sl = spool.tile([128, cs], F32, tag="silu")
                    nc.scalar.activation(
                        sl[:], a1[:], mybir.ActivationFunctionType.Silu
                    )
                    hh = hpool.tile([128, cs], BF16, tag=f"hh{ib}")
                    nc.vector.tensor_mul(hh[:], sl[:], a2[:])
                    hhs.append(hh)

                # stage 2: Y[c, h] = Hh @ D^T  (contract I)
                for h0 in range(0, H, 512):
                    for cs0 in range(0, cs, 128):
                        w = min(128, cs - cs0)
                        py = psum.tile([w, 512], F32, tag="py")
                        for ic in range(IB):
                            nc.tensor.matmul(
                                py[:],
                                hhs[ic][:, cs0 : cs0 + w],
                                dts[ic][:, h0 : h0 + 512],
                                start=(ic == 0), stop=(ic == IB - 1),
                            )
                        yt = ypool.tile([w, 512], F32, tag="yt")
                        nc.scalar.copy(yt[:], py[:])
                        nc.sync.dma_start(
                            y_d[c0 + cs0 : c0 + cs0 + w, h0 : h0 + 512], yt[:]
                        )



def _routing(x, router_w):
    """Replicate the reference's routing decisions with identical jax ops."""
    import jax
    import jax.numpy as jnp

    xf = jnp.asarray(x).reshape(-1, H)
    logits = xf @ jnp.asarray(router_w).T
    probs = jax.nn.softmax(logits, axis=-1)
    topk_p, topk_i = jax.lax.top_k(probs, TOPK)
    topk_p = topk_p / topk_p.sum(axis=-1, keepdims=True)
    return np.asarray(topk_p), np.asarray(topk_i)


def prepare(x, router_w, gate_w, up_w, down_w):
    """Host-side dispatch: returns (nc, in_maps, combine) where combine maps
    the per-core device outputs to the full [B,S,H] result."""
    topk_p, topk_i = _routing(x, router_w)
    xf = np.ascontiguousarray(np.asarray(x, dtype=np.float32).reshape(T, H))

    idxs, weights = [], []
    for e in range(E):
        sel = topk_i == e
        mask = sel.any(axis=-1)
        w_tok = (topk_p * sel).sum(axis=-1)
        cnt = int(mask.sum())
        mean_w = float(w_tok.sum() / max(cnt, 1)) if cnt > 0 else 0.0
        idxs.append(np.nonzero(mask)[0])
        weights.append(np.float32(mean_w))

    cmax = max(len(ix) for ix in idxs)
    C = ((cmax + 127) // 128) * 128

    xf_bf = xf.astype(ml_dtypes.bfloat16)
    in_maps = []
    for e in range(E):
        ix = idxs[e]
        # X^T packed as [KCH, 128, C]: partition p of chunk k holds row h=k*128+p
        xt = np.zeros((KCH, 128, C), dtype=ml_dtypes.bfloat16)
        xt[:, :, : len(ix)] = xf_bf[ix].T.reshape(KCH, 128, len(ix))
        # G/U packed as [IB, 128(p over H%128? no: p over H), KCH, 128(i)]
        # G^T is [H, I]; tile (ib): [H, 128] -> [128(p), KCH(k), 128(i)] with h = k*128+p
        gT = np.asarray(gate_w[e], dtype=np.float32).T.astype(ml_dtypes.bfloat16)
        uT = np.asarray(up_w[e], dtype=np.float32).T.astype(ml_dtypes.bfloat16)
        gt = np.ascontiguousarray(
            gT.reshape(KCH, 128, IB, 128).transpose(2, 1, 0, 3)
        )
        ut = np.ascontiguousarray(
            uT.reshape(KCH, 128, IB, 128).transpose(2, 1, 0, 3)
        )
        # D^T is [I, H]; packed [IB, 128(p over I), H] with i = ic*128+p
        dT = np.asarray(down_w[e], dtype=np.float32).T.astype(ml_dtypes.bfloat16)
        dt = np.ascontiguousarray(dT.reshape(IB, 128, H))
        in_maps.append({"xt": xt, "gt": gt, "ut": ut, "dt": dt})

    nc = build_program(C)

    def combine(results):
        out = np.zeros((T, H), dtype=np.float32)
        for e in range(E):
            ix = idxs[e]
            y = results[e]["y"]
            out[ix] += weights[e] * y[: len(ix)]
        return out.reshape(B, S, H)

    return nc, in_maps, combine


def kernel(x, router_w, gate_w, up_w, down_w):
    nc, in_maps, combine = prepare(x, router_w, gate_w, up_w, down_w)
    res = run_bass_kernel_spmd(nc, in_maps, list(range(8)))
    return combine(res.results)
